# revision 12
# baseline (speedup 1.0000x reference)
import sys

if "/opt/trn_rl_repo" not in sys.path:
    sys.path.insert(0, "/opt/trn_rl_repo")

from contextlib import ExitStack

import numpy as np
import ml_dtypes

import concourse.bass as bass
import concourse.tile as tile
import concourse.mybir as mybir
from concourse import bacc
from concourse import bass_utils
from concourse.masks import make_identity

F32 = mybir.dt.float32
F32R = mybir.dt.float32r
BF16 = mybir.dt.bfloat16

# Problem constants (hardcoded per contest contract)
BS = 16           # total batches
NCORES = 8
BPC = BS // NCORES  # batches per core = 2
AL = 1024         # A_len
PL = 2048         # P_len
L = 768           # feature dim
KC = L // 128     # 6 contraction chunks for l/m dims
ACH = AL // 128   # 8 a-chunks
PCH = PL // 128   # 16 p-chunks
PB = 512          # p block size
NPB = PL // PB    # 4 p blocks
PBC = PB // 128   # 4 p-chunks per block
K2 = 2 * L // 128  # 12 chunks of concat dim


def _f32r(ap):
    return ap.bitcast(F32R)


def build_program(trace_sim=False):
    """Build the per-core Bass program: processes BPC batches."""
    nc = bacc.Bacc("TRN2", target_bir_lowering=False, debug=False)

    ATd = nc.dram_tensor("AT", [BPC, L, AL], F32R, kind="ExternalInput")
    PTd = nc.dram_tensor("PT", [BPC, L, PL], F32R, kind="ExternalInput")
    PNd = nc.dram_tensor("PN", [BPC, PL, L], BF16, kind="ExternalInput")
    GWd = nc.dram_tensor("GW", [L, L], F32R, kind="ExternalInput")
    GBd = nc.dram_tensor("GB", [L], F32, kind="ExternalInput")
    FCAWd = nc.dram_tensor("FCAW", [2 * L, L], BF16, kind="ExternalInput")
    FCABd = nc.dram_tensor("FCAB", [L], F32, kind="ExternalInput")
    FCPWd = nc.dram_tensor("FCPW", [2 * L, L], BF16, kind="ExternalInput")
    FCPBd = nc.dram_tensor("FCPB", [L], F32, kind="ExternalInput")
    SAd = nc.dram_tensor("SA", [BPC, AL, L], F32, kind="ExternalOutput")
    SPd = nc.dram_tensor("SP", [BPC, PL, L], F32, kind="ExternalOutput")

    with tile.TileContext(nc, trace_sim=trace_sim) as tc:
        with ExitStack() as ctx:
            singles = ctx.enter_context(tc.tile_pool(name="singles", bufs=1))
            # shared-lifetime big slots
            sh24 = ctx.enter_context(tc.tile_pool(name="sh24", bufs=1))
            sh24b = ctx.enter_context(tc.tile_pool(name="sh24b", bufs=1))
            perb = ctx.enter_context(tc.tile_pool(name="perb", bufs=1))
            blk2 = ctx.enter_context(tc.tile_pool(name="blk2", bufs=2))
            blk1 = ctx.enter_context(tc.tile_pool(name="blk1", bufs=1))
            stats = ctx.enter_context(tc.tile_pool(name="stats", bufs=4))
            stage = ctx.enter_context(tc.tile_pool(name="stage", bufs=2))
            psA = ctx.enter_context(tc.tile_pool(name="psA", bufs=2, space="PSUM"))
            psW = ctx.enter_context(tc.tile_pool(name="psW", bufs=1, space="PSUM"))
            psTM = ctx.enter_context(tc.tile_pool(name="psTM", bufs=2, space="PSUM"))
            rr = [0]

            def big_psum(dt=F32):
                rr[0] += 1
                pool = psW if rr[0] % 3 == 0 else psA
                return pool.tile([128, AL], dt, name="bps",
                                 tag="a" if pool is psA else "w")

            # --- batch-0 critical-path inputs first (gates first matmul) ---
            at0 = sh24.tile([128, KC, AL], F32R, tag="t24")
            for k in range(KC):
                nc.sync.dma_start(at0[:, k, :], ATd[0, k * 128:(k + 1) * 128, :])
            gw0 = sh24b.tile([128, KC, L], F32R, tag="t24b")
            for k in range(KC):
                nc.sync.dma_start(gw0[:, k, :], GWd[k * 128:(k + 1) * 128, :])

            # --- resident weights (DMAs for the big FC weights are issued
            # after batch-0 phase A so they don't steal bandwidth from the
            # critical-path AT/GW/PT loads) ---
            fcaw = singles.tile([128, K2, L], BF16)
            fcpw = singles.tile([128, K2, L], BF16)
            fcab = singles.tile([128, L], F32)
            fcpb = singles.tile([128, L], F32)
            def bcast128(d):
                a = d.ap()
                return bass.AP(tensor=a.tensor, offset=a.offset,
                               ap=[[0, 128]] + list(a.ap))
            nc.sync.dma_start(fcab[:], bcast128(FCABd))
            nc.sync.dma_start(fcpb[:], bcast128(FCPBd))
            gb = singles.tile([128, KC], F32)
            nc.sync.dma_start(gb[:], GBd.ap().rearrange("(c p) -> p c", p=128))
            id32f = singles.tile([128, 128], F32)
            make_identity(nc, id32f[:])
            id32 = singles.tile([128, 128], F32R)
            nc.vector.tensor_copy(id32[:], id32f[:])
            id16 = singles.tile([128, 128], BF16)
            make_identity(nc, id16[:])

            for b in range(BPC):
                # ---------- Phase A: AgT[m, a] = (G_w.T @ A.T) + G_b ----------
                if b == 0:
                    at, gw = at0, gw0
                else:
                    at = sh24.tile([128, KC, AL], F32R, tag="t24")
                    for k in range(KC):
                        nc.sync.dma_start(at[:, k, :], ATd[b, k * 128:(k + 1) * 128, :])
                    gw = sh24b.tile([128, KC, L], F32R, tag="t24b")
                    for k in range(KC):
                        nc.sync.dma_start(gw[:, k, :], GWd[k * 128:(k + 1) * 128, :])

                agt = perb.tile([128, KC, AL], F32R, tag="agt")
                for mi in range(KC):
                    pw = big_psum()
                    for h in range(2):
                        for k in range(KC):
                            nc.tensor.matmul(
                                pw[:, h * 512:(h + 1) * 512],
                                gw[:, k, mi * 128:(mi + 1) * 128],
                                at[:, k, h * 512:(h + 1) * 512],
                                start=(k == 0), stop=(k == KC - 1))
                    # add G_b (per-partition) while copying PSUM -> SBUF
                    nc.scalar.add(agt[:, mi, :], pw[:], gb[:, mi:mi + 1])

                if b == 0:
                    for k in range(K2):
                        nc.sync.dma_start(fcpw[:, k, :], FCPWd[k * 128:(k + 1) * 128, :])
                        nc.sync.dma_start(fcaw[:, k, :], FCAWd[k * 128:(k + 1) * 128, :])

                # ---------- Phase B: Ag[a, m] bf16 via PE transpose ----------
                ag = perb.tile([128, ACH, L], BF16, tag="ag")
                for ai in range(ACH):
                    pt_ = big_psum(F32R)
                    for mi in range(KC):
                        nc.tensor.transpose(
                            pt_[:, mi * 128:(mi + 1) * 128],
                            agt[:, mi, ai * 128:(ai + 1) * 128], id32[:])
                    nc.vector.tensor_copy(ag[:, ai, :], pt_[:, :L])

                # ---------- Phase C: stream over p blocks ----------
                mat = sh24.tile([128, KC, AL], F32R, tag="t24")
                for pb in range(NPB):
                    ptb = blk2.tile([128, KC, PB], F32R, tag="ptb")
                    for k in range(KC):
                        nc.sync.dma_start(
                            ptb[:, k, :],
                            PTd[b, k * 128:(k + 1) * 128, pb * PB:(pb + 1) * PB])
                    pnb = blk2.tile([128, PBC, L], BF16, tag="pnb")
                    for pi in range(PBC):
                        p0 = pb * PB + pi * 128
                        nc.sync.dma_start(pnb[:, pi, :], PNd[b, p0:p0 + 128, :])

                    wn = blk2.tile([128, PBC, AL], BF16, tag="wn")
                    # scores + softmax per p-chunk
                    for pi in range(PBC):
                        pw = big_psum()
                        for h in range(2):
                            for k in range(KC):
                                nc.tensor.matmul(
                                    pw[:, h * 512:(h + 1) * 512],
                                    ptb[:, k, pi * 128:(pi + 1) * 128],
                                    agt[:, k, h * 512:(h + 1) * 512],
                                    start=(k == 0), stop=(k == KC - 1))
                        nm0 = stats.tile([128, 1], F32, tag="nm0")
                        nm1 = stats.tile([128, 1], F32, tag="nm1")
                        nc.vector.tensor_reduce(
                            nm0[:], pw[:, 0:512], axis=mybir.AxisListType.X,
                            op=mybir.AluOpType.max, negate=True)
                        nc.vector.tensor_reduce(
                            nm1[:], pw[:, 512:AL], axis=mybir.AxisListType.X,
                            op=mybir.AluOpType.max, negate=True)
                        negmax = stats.tile([128, 1], F32, tag="negmax")
                        nc.vector.tensor_tensor(
                            negmax[:], nm0[:], nm1[:], op=mybir.AluOpType.min)
                        rsum = stats.tile([128, 1], F32, tag="rsum")
                        nc.scalar.activation(
                            wn[:, pi, :], pw[:],
                            mybir.ActivationFunctionType.Exp,
                            bias=negmax[:], scale=1.0, accum_out=rsum[:])
                        rinv = stats.tile([128, 1], F32, tag="rinv")
                        nc.vector.reciprocal(rinv[:], rsum[:])
                        nc.vector.tensor_scalar_mul(wn[:, pi, :], wn[:, pi, :], rinv[:])

                    # transpose Wn -> WnT [a, p_block]
                    wnt = blk1.tile([128, ACH, PB], BF16, tag="wnt")
                    for ai in range(ACH):
                        ptr = psTM.tile([128, PB], BF16, tag="tm")
                        for pi in range(PBC):
                            nc.tensor.transpose(
                                ptr[:, pi * 128:(pi + 1) * 128],
                                wn[:, pi, ai * 128:(ai + 1) * 128], id16[:])
                        nc.vector.tensor_copy(wnt[:, ai, :], ptr[:])

                    # MPT[m, p_blk] = Ag.T @ WnT ; SPcatT rows
                    spcat = blk1.tile([128, K2, PB], BF16, tag="spcat")
                    for mi in range(KC):
                        pm = psTM.tile([128, PB], F32, tag="tm")
                        for ai in range(ACH):
                            nc.tensor.matmul(
                                pm[:], ag[:, ai, mi * 128:(mi + 1) * 128],
                                wnt[:, ai, :],
                                start=(ai == 0), stop=(ai == ACH - 1))
                        nc.vector.tensor_sub(spcat[:, mi, :], pm[:], ptb[:, mi, :])
                        nc.vector.tensor_mul(spcat[:, KC + mi, :], pm[:], ptb[:, mi, :])

                    # SP_out rows for this block
                    for pi in range(PBC):
                        ps = big_psum()
                        for h, (n0, n1) in enumerate(((0, 512), (512, L))):
                            for k in range(K2):
                                nc.tensor.matmul(
                                    ps[:, n0:n1],
                                    spcat[:, k, pi * 128:(pi + 1) * 128],
                                    fcpw[:, k, n0:n1],
                                    start=(k == 0), stop=(k == K2 - 1))
                        so = stage.tile([128, L], F32, tag="st")
                        nc.vector.tensor_add(so[:], ps[:, :L], fcpb[:])
                        nc.scalar.activation(so[:], so[:],
                                             mybir.ActivationFunctionType.Relu)
                        p0 = pb * PB + pi * 128
                        nc.gpsimd.dma_start(SPd[b, p0:p0 + 128, :], so[:])

                    # MAT partial: mat[l, a] += P_blk.T-contract @ Wn_blk
                    for li in range(KC):
                        pa = big_psum()
                        for h in range(2):
                            for pi in range(PBC):
                                nc.tensor.matmul(
                                    pa[:, h * 512:(h + 1) * 512],
                                    pnb[:, pi, li * 128:(li + 1) * 128],
                                    wn[:, pi, h * 512:(h + 1) * 512],
                                    start=(pi == 0), stop=(pi == PBC - 1))
                        if pb == 0:
                            nc.vector.tensor_copy(mat[:, li, :], pa[:])
                        else:
                            nc.vector.tensor_add(mat[:, li, :], pa[:], mat[:, li, :])

                # ---------- Phase D: SAcatT + fca ----------
                sacat = sh24b.tile([128, K2, AL], BF16, tag="t24b")
                for li in range(KC):
                    nc.vector.tensor_sub(sacat[:, li, :], mat[:, li, :], agt[:, li, :])
                    nc.vector.tensor_mul(sacat[:, KC + li, :], mat[:, li, :], agt[:, li, :])

                for ai in range(ACH):
                    ps = big_psum()
                    for h, (n0, n1) in enumerate(((0, 512), (512, L))):
                        for k in range(K2):
                            nc.tensor.matmul(
                                ps[:, n0:n1],
                                sacat[:, k, ai * 128:(ai + 1) * 128],
                                fcaw[:, k, n0:n1],
                                start=(k == 0), stop=(k == K2 - 1))
                    so = stage.tile([128, L], F32, tag="st")
                    nc.vector.tensor_add(so[:], ps[:, :L], fcab[:])
                    nc.scalar.activation(so[:], so[:],
                                         mybir.ActivationFunctionType.Relu)
                    nc.gpsimd.dma_start(SAd[b, ai * 128:(ai + 1) * 128, :], so[:])

    nc.compile()
    return nc


_NC_CACHE = {}


def _get_program():
    if "nc" not in _NC_CACHE:
        _NC_CACHE["nc"] = build_program()
    return _NC_CACHE["nc"]


def kernel(A, P, G_w, G_b, fca_w, fca_b, fcp_w, fcp_b):
    A = np.asarray(A, dtype=np.float32)
    P = np.asarray(P, dtype=np.float32)
    G_w = np.ascontiguousarray(np.asarray(G_w, dtype=np.float32))
    G_b = np.ascontiguousarray(np.asarray(G_b, dtype=np.float32))
    fca_w16 = np.asarray(fca_w, dtype=np.float32).astype(ml_dtypes.bfloat16)
    fcp_w16 = np.asarray(fcp_w, dtype=np.float32).astype(ml_dtypes.bfloat16)
    fca_b = np.ascontiguousarray(np.asarray(fca_b, dtype=np.float32))
    fcp_b = np.ascontiguousarray(np.asarray(fcp_b, dtype=np.float32))

    AT = np.ascontiguousarray(A.transpose(0, 2, 1))      # [16, 768, 1024]
    PT = np.ascontiguousarray(P.transpose(0, 2, 1))      # [16, 768, 2048]
    PN = P.astype(ml_dtypes.bfloat16)                     # [16, 2048, 768]

    nc = _get_program()
    in_maps = []
    for c in range(NCORES):
        b0, b1 = c * BPC, (c + 1) * BPC
        in_maps.append({
            "AT": np.ascontiguousarray(AT[b0:b1]),
            "PT": np.ascontiguousarray(PT[b0:b1]),
            "PN": np.ascontiguousarray(PN[b0:b1]),
            "GW": G_w, "GB": G_b,
            "FCAW": fca_w16, "FCAB": fca_b,
            "FCPW": fcp_w16, "FCPB": fcp_b,
        })
    res = bass_utils.run_bass_kernel_spmd(nc, in_maps, core_ids=list(range(NCORES)))
    SA = np.concatenate([r["SA"] for r in res.results], axis=0)
    SP = np.concatenate([r["SP"] for r in res.results], axis=0)
    return (SA, SP)


# revision 13
# speedup vs baseline: 1.2003x; 1.2003x over previous
import sys

if "/opt/trn_rl_repo" not in sys.path:
    sys.path.insert(0, "/opt/trn_rl_repo")

from contextlib import ExitStack

import numpy as np
import ml_dtypes

import concourse.bass as bass
import concourse.tile as tile
import concourse.mybir as mybir
from concourse import bacc
from concourse import bass_utils
from concourse.masks import make_identity

F32 = mybir.dt.float32
F32R = mybir.dt.float32r
BF16 = mybir.dt.bfloat16

# Problem constants (hardcoded per contest contract)
BS = 16           # total batches
NCORES = 8
BPC = BS // NCORES  # batches per core = 2
AL = 1024         # A_len
PL = 2048         # P_len
L = 768           # feature dim
KC = L // 128     # 6 contraction chunks for l/m dims
ACH = AL // 128   # 8 a-chunks
PCH = PL // 128   # 16 p-chunks
PB = 512          # p block size
NPB = PL // PB    # 4 p blocks
PBC = PB // 128   # 4 p-chunks per block
K2 = 2 * L // 128  # 12 chunks of concat dim


def _f32r(ap):
    return ap.bitcast(F32R)


def build_program(trace_sim=False):
    """Build the per-core Bass program: processes BPC batches."""
    nc = bacc.Bacc("TRN2", target_bir_lowering=False, debug=False)

    ATd = nc.dram_tensor("AT", [BPC, L, AL], F32R, kind="ExternalInput")
    PTd = nc.dram_tensor("PT", [BPC, L, PL], F32R, kind="ExternalInput")
    PNd = nc.dram_tensor("PN", [BPC, PL, L], BF16, kind="ExternalInput")
    GWd = nc.dram_tensor("GW", [L, L], F32R, kind="ExternalInput")
    GBd = nc.dram_tensor("GB", [L], F32, kind="ExternalInput")
    FCAWd = nc.dram_tensor("FCAW", [2 * L, L], BF16, kind="ExternalInput")
    FCABd = nc.dram_tensor("FCAB", [L], F32, kind="ExternalInput")
    FCPWd = nc.dram_tensor("FCPW", [2 * L, L], BF16, kind="ExternalInput")
    FCPBd = nc.dram_tensor("FCPB", [L], F32, kind="ExternalInput")
    SAd = nc.dram_tensor("SA", [BPC, AL, L], F32, kind="ExternalOutput")
    SPd = nc.dram_tensor("SP", [BPC, PL, L], F32, kind="ExternalOutput")

    with tile.TileContext(nc, trace_sim=trace_sim) as tc:
        with ExitStack() as ctx:
            singles = ctx.enter_context(tc.tile_pool(name="singles", bufs=1))
            # shared-lifetime big slots
            sh24 = ctx.enter_context(tc.tile_pool(name="sh24", bufs=1))
            sh24b = ctx.enter_context(tc.tile_pool(name="sh24b", bufs=1))
            perb = ctx.enter_context(tc.tile_pool(name="perb", bufs=1))
            blk2 = ctx.enter_context(tc.tile_pool(name="blk2", bufs=2))
            blk1 = ctx.enter_context(tc.tile_pool(name="blk1", bufs=1))
            stats = ctx.enter_context(tc.tile_pool(name="stats", bufs=4))
            stage = ctx.enter_context(tc.tile_pool(name="stage", bufs=2))
            psA = ctx.enter_context(tc.tile_pool(name="psA", bufs=2, space="PSUM"))
            psW = ctx.enter_context(tc.tile_pool(name="psW", bufs=1, space="PSUM"))
            psTM = ctx.enter_context(tc.tile_pool(name="psTM", bufs=2, space="PSUM"))
            rr = [0]

            def big_psum(dt=F32):
                rr[0] += 1
                pool = psW if rr[0] % 3 == 0 else psA
                return pool.tile([128, AL], dt, name="bps",
                                 tag="a" if pool is psA else "w")

            # --- batch-0 critical-path inputs first (gates first matmul) ---
            at0 = sh24.tile([128, KC, AL], F32R, tag="t24")
            for k in range(KC):
                nc.sync.dma_start(at0[:, k, :], ATd[0, k * 128:(k + 1) * 128, :])
            gw0 = sh24b.tile([128, KC, L], F32R, tag="t24b")
            for k in range(KC):
                nc.sync.dma_start(gw0[:, k, :], GWd[k * 128:(k + 1) * 128, :])

            # --- resident weights (DMAs for the big FC weights are issued
            # after batch-0 phase A so they don't steal bandwidth from the
            # critical-path AT/GW/PT loads) ---
            fcaw = singles.tile([128, K2, L], BF16)
            fcpw = singles.tile([128, K2, L], BF16)
            fcab = singles.tile([128, L], F32)
            fcpb = singles.tile([128, L], F32)
            def bcast128(d):
                a = d.ap()
                return bass.AP(tensor=a.tensor, offset=a.offset,
                               ap=[[0, 128]] + list(a.ap))
            nc.sync.dma_start(fcab[:], bcast128(FCABd))
            nc.sync.dma_start(fcpb[:], bcast128(FCPBd))
            gb = singles.tile([128, KC], F32)
            nc.sync.dma_start(gb[:], GBd.ap().rearrange("(c p) -> p c", p=128))
            id32f = singles.tile([128, 128], F32)
            make_identity(nc, id32f[:])
            id32 = singles.tile([128, 128], F32R)
            nc.vector.tensor_copy(id32[:], id32f[:])
            id16 = singles.tile([128, 128], BF16)
            make_identity(nc, id16[:])

            for b in range(BPC):
                # ---------- Phase A: AgT[m, a] = (G_w.T @ A.T) + G_b ----------
                if b == 0:
                    at, gw = at0, gw0
                else:
                    at = sh24.tile([128, KC, AL], F32R, tag="t24")
                    for k in range(KC):
                        nc.sync.dma_start(at[:, k, :], ATd[b, k * 128:(k + 1) * 128, :])
                    gw = sh24b.tile([128, KC, L], F32R, tag="t24b")
                    for k in range(KC):
                        nc.sync.dma_start(gw[:, k, :], GWd[k * 128:(k + 1) * 128, :])

                agt = perb.tile([128, KC, AL], F32R, tag="agt")
                for mi in range(KC):
                    pw = big_psum()
                    for h in range(2):
                        for k in range(KC):
                            nc.tensor.matmul(
                                pw[:, h * 512:(h + 1) * 512],
                                gw[:, k, mi * 128:(mi + 1) * 128],
                                at[:, k, h * 512:(h + 1) * 512],
                                start=(k == 0), stop=(k == KC - 1))
                    # add G_b (per-partition) while copying PSUM -> SBUF
                    nc.scalar.add(agt[:, mi, :], pw[:], gb[:, mi:mi + 1])

                if b == 0:
                    for k in range(K2):
                        nc.sync.dma_start(fcpw[:, k, :], FCPWd[k * 128:(k + 1) * 128, :])
                        nc.sync.dma_start(fcaw[:, k, :], FCAWd[k * 128:(k + 1) * 128, :])

                # ---------- Phase B: Ag[a, m] bf16 via PE transpose ----------
                ag = perb.tile([128, ACH, L], BF16, tag="ag")
                for ai in range(ACH):
                    pt_ = big_psum(F32R)
                    for mi in range(KC):
                        nc.tensor.transpose(
                            pt_[:, mi * 128:(mi + 1) * 128],
                            agt[:, mi, ai * 128:(ai + 1) * 128], id32[:])
                    nc.vector.tensor_copy(ag[:, ai, :], pt_[:, :L])

                # ---------- Phase C: stream over p blocks ----------
                mat = sh24.tile([128, KC, AL], F32R, tag="t24")
                for pb in range(NPB):
                    ptb = blk2.tile([128, KC, PB], F32R, tag="ptb")
                    for k in range(KC):
                        nc.sync.dma_start(
                            ptb[:, k, :],
                            PTd[b, k * 128:(k + 1) * 128, pb * PB:(pb + 1) * PB])
                    pnb = blk2.tile([128, PBC, L], BF16, tag="pnb")
                    for pi in range(PBC):
                        p0 = pb * PB + pi * 128
                        nc.sync.dma_start(pnb[:, pi, :], PNd[b, p0:p0 + 128, :])

                    wn = blk2.tile([128, PBC, AL], BF16, tag="wn")
                    # scores + softmax per p-chunk
                    for pi in range(PBC):
                        pw = big_psum()
                        for h in range(2):
                            for k in range(KC):
                                nc.tensor.matmul(
                                    pw[:, h * 512:(h + 1) * 512],
                                    ptb[:, k, pi * 128:(pi + 1) * 128],
                                    agt[:, k, h * 512:(h + 1) * 512],
                                    start=(k == 0), stop=(k == KC - 1))
                        nm0 = stats.tile([128, 1], F32, tag="nm0")
                        nm1 = stats.tile([128, 1], F32, tag="nm1")
                        nc.vector.tensor_reduce(
                            nm0[:], pw[:, 0:512], axis=mybir.AxisListType.X,
                            op=mybir.AluOpType.max, negate=True)
                        nc.vector.tensor_reduce(
                            nm1[:], pw[:, 512:AL], axis=mybir.AxisListType.X,
                            op=mybir.AluOpType.max, negate=True)
                        negmax = stats.tile([128, 1], F32, tag="negmax")
                        nc.vector.tensor_tensor(
                            negmax[:], nm0[:], nm1[:], op=mybir.AluOpType.min)
                        rsum = stats.tile([128, 1], F32, tag="rsum")
                        nc.scalar.activation(
                            wn[:, pi, :], pw[:],
                            mybir.ActivationFunctionType.Exp,
                            bias=negmax[:], scale=1.0, accum_out=rsum[:])
                        rinv = stats.tile([128, 1], F32, tag="rinv")
                        nc.vector.reciprocal(rinv[:], rsum[:])
                        nc.vector.tensor_scalar_mul(wn[:, pi, :], wn[:, pi, :], rinv[:])

                    # transpose Wn -> WnT [a, p_block]
                    wnt = blk1.tile([128, ACH, PB], BF16, tag="wnt")
                    for ai in range(ACH):
                        ptr = psTM.tile([128, PB], BF16, tag="tm")
                        for pi in range(PBC):
                            nc.tensor.transpose(
                                ptr[:, pi * 128:(pi + 1) * 128],
                                wn[:, pi, ai * 128:(ai + 1) * 128], id16[:])
                        nc.vector.tensor_copy(wnt[:, ai, :], ptr[:])

                    # MPT[m, p_blk] = Ag.T @ WnT ; SPcatT rows
                    spcat = blk1.tile([128, K2, PB], BF16, tag="spcat")
                    for mi in range(KC):
                        pm = psTM.tile([128, PB], F32, tag="tm")
                        for ai in range(ACH):
                            nc.tensor.matmul(
                                pm[:], ag[:, ai, mi * 128:(mi + 1) * 128],
                                wnt[:, ai, :],
                                start=(ai == 0), stop=(ai == ACH - 1))
                        nc.vector.tensor_sub(spcat[:, mi, :], pm[:], ptb[:, mi, :])
                        nc.vector.tensor_mul(spcat[:, KC + mi, :], pm[:], ptb[:, mi, :])

                    # SP_out rows for this block
                    for pi in range(PBC):
                        ps = big_psum()
                        for h, (n0, n1) in enumerate(((0, 512), (512, L))):
                            for k in range(K2):
                                nc.tensor.matmul(
                                    ps[:, n0:n1],
                                    spcat[:, k, pi * 128:(pi + 1) * 128],
                                    fcpw[:, k, n0:n1],
                                    start=(k == 0), stop=(k == K2 - 1))
                        so = stage.tile([128, L], F32, tag="st")
                        nc.vector.tensor_add(so[:], ps[:, :L], fcpb[:])
                        nc.scalar.activation(so[:], so[:],
                                             mybir.ActivationFunctionType.Relu)
                        p0 = pb * PB + pi * 128
                        nc.scalar.dma_start(SPd[b, p0:p0 + 128, :], so[:])

                    # MAT partial: mat[l, a] += P_blk.T-contract @ Wn_blk
                    for li in range(KC):
                        pa = big_psum()
                        for h in range(2):
                            for pi in range(PBC):
                                nc.tensor.matmul(
                                    pa[:, h * 512:(h + 1) * 512],
                                    pnb[:, pi, li * 128:(li + 1) * 128],
                                    wn[:, pi, h * 512:(h + 1) * 512],
                                    start=(pi == 0), stop=(pi == PBC - 1))
                        if pb == 0:
                            nc.vector.tensor_copy(mat[:, li, :], pa[:])
                        else:
                            nc.vector.tensor_add(mat[:, li, :], pa[:], mat[:, li, :])

                # ---------- Phase D: SAcatT + fca ----------
                sacat = sh24b.tile([128, K2, AL], BF16, tag="t24b")
                for li in range(KC):
                    nc.vector.tensor_sub(sacat[:, li, :], mat[:, li, :], agt[:, li, :])
                    nc.vector.tensor_mul(sacat[:, KC + li, :], mat[:, li, :], agt[:, li, :])

                for ai in range(ACH):
                    ps = big_psum()
                    for h, (n0, n1) in enumerate(((0, 512), (512, L))):
                        for k in range(K2):
                            nc.tensor.matmul(
                                ps[:, n0:n1],
                                sacat[:, k, ai * 128:(ai + 1) * 128],
                                fcaw[:, k, n0:n1],
                                start=(k == 0), stop=(k == K2 - 1))
                    so = stage.tile([128, L], F32, tag="st")
                    nc.vector.tensor_add(so[:], ps[:, :L], fcab[:])
                    nc.scalar.activation(so[:], so[:],
                                         mybir.ActivationFunctionType.Relu)
                    nc.scalar.dma_start(SAd[b, ai * 128:(ai + 1) * 128, :], so[:])

    nc.compile()
    return nc


_NC_CACHE = {}


def _get_program():
    if "nc" not in _NC_CACHE:
        _NC_CACHE["nc"] = build_program()
    return _NC_CACHE["nc"]


def kernel(A, P, G_w, G_b, fca_w, fca_b, fcp_w, fcp_b):
    A = np.asarray(A, dtype=np.float32)
    P = np.asarray(P, dtype=np.float32)
    G_w = np.ascontiguousarray(np.asarray(G_w, dtype=np.float32))
    G_b = np.ascontiguousarray(np.asarray(G_b, dtype=np.float32))
    fca_w16 = np.asarray(fca_w, dtype=np.float32).astype(ml_dtypes.bfloat16)
    fcp_w16 = np.asarray(fcp_w, dtype=np.float32).astype(ml_dtypes.bfloat16)
    fca_b = np.ascontiguousarray(np.asarray(fca_b, dtype=np.float32))
    fcp_b = np.ascontiguousarray(np.asarray(fcp_b, dtype=np.float32))

    AT = np.ascontiguousarray(A.transpose(0, 2, 1))      # [16, 768, 1024]
    PT = np.ascontiguousarray(P.transpose(0, 2, 1))      # [16, 768, 2048]
    PN = P.astype(ml_dtypes.bfloat16)                     # [16, 2048, 768]

    nc = _get_program()
    in_maps = []
    for c in range(NCORES):
        b0, b1 = c * BPC, (c + 1) * BPC
        in_maps.append({
            "AT": np.ascontiguousarray(AT[b0:b1]),
            "PT": np.ascontiguousarray(PT[b0:b1]),
            "PN": np.ascontiguousarray(PN[b0:b1]),
            "GW": G_w, "GB": G_b,
            "FCAW": fca_w16, "FCAB": fca_b,
            "FCPW": fcp_w16, "FCPB": fcp_b,
        })
    res = bass_utils.run_bass_kernel_spmd(nc, in_maps, core_ids=list(range(NCORES)))
    SA = np.concatenate([r["SA"] for r in res.results], axis=0)
    SP = np.concatenate([r["SP"] for r in res.results], axis=0)
    return (SA, SP)


# revision 14
# speedup vs baseline: 1.2721x; 1.0598x over previous
import sys

if "/opt/trn_rl_repo" not in sys.path:
    sys.path.insert(0, "/opt/trn_rl_repo")

from contextlib import ExitStack

import numpy as np
import ml_dtypes

import concourse.bass as bass
import concourse.tile as tile
import concourse.mybir as mybir
from concourse import bacc
from concourse import bass_utils
from concourse.masks import make_identity

F32 = mybir.dt.float32
F32R = mybir.dt.float32r
BF16 = mybir.dt.bfloat16

# Problem constants (hardcoded per contest contract)
BS = 16           # total batches
NCORES = 8
BPC = BS // NCORES  # batches per core = 2
AL = 1024         # A_len
PL = 2048         # P_len
L = 768           # feature dim
KC = L // 128     # 6 contraction chunks for l/m dims
ACH = AL // 128   # 8 a-chunks
PCH = PL // 128   # 16 p-chunks
PB = 512          # p block size
NPB = PL // PB    # 4 p blocks
PBC = PB // 128   # 4 p-chunks per block
K2 = 2 * L // 128  # 12 chunks of concat dim


def _f32r(ap):
    return ap.bitcast(F32R)


def build_program(trace_sim=False):
    """Build the per-core Bass program: processes BPC batches."""
    nc = bacc.Bacc("TRN2", target_bir_lowering=False, debug=False)

    ATd = nc.dram_tensor("AT", [BPC, L, AL], F32R, kind="ExternalInput")
    PTd = nc.dram_tensor("PT", [BPC, L, PL], F32R, kind="ExternalInput")
    PNd = nc.dram_tensor("PN", [BPC, PL, L], BF16, kind="ExternalInput")
    GWd = nc.dram_tensor("GW", [L, L], F32R, kind="ExternalInput")
    GBd = nc.dram_tensor("GB", [L], F32, kind="ExternalInput")
    FCAWd = nc.dram_tensor("FCAW", [2 * L, L], BF16, kind="ExternalInput")
    FCABd = nc.dram_tensor("FCAB", [L], F32, kind="ExternalInput")
    FCPWd = nc.dram_tensor("FCPW", [2 * L, L], BF16, kind="ExternalInput")
    FCPBd = nc.dram_tensor("FCPB", [L], F32, kind="ExternalInput")
    SAd = nc.dram_tensor("SA", [BPC, AL, L], F32, kind="ExternalOutput")
    SPd = nc.dram_tensor("SP", [BPC, PL, L], F32, kind="ExternalOutput")

    with tile.TileContext(nc, trace_sim=trace_sim) as tc:
        with ExitStack() as ctx:
            singles = ctx.enter_context(tc.tile_pool(name="singles", bufs=1))
            # shared-lifetime big slots
            sh24 = ctx.enter_context(tc.tile_pool(name="sh24", bufs=1))
            sh24b = ctx.enter_context(tc.tile_pool(name="sh24b", bufs=1))
            perb = ctx.enter_context(tc.tile_pool(name="perb", bufs=1))
            blk2 = ctx.enter_context(tc.tile_pool(name="blk2", bufs=2))
            blk1 = ctx.enter_context(tc.tile_pool(name="blk1", bufs=1))
            stats = ctx.enter_context(tc.tile_pool(name="stats", bufs=4))
            stage = ctx.enter_context(tc.tile_pool(name="stage", bufs=2))
            psA = ctx.enter_context(tc.tile_pool(name="psA", bufs=2, space="PSUM"))
            psW = ctx.enter_context(tc.tile_pool(name="psW", bufs=1, space="PSUM"))
            psTM = ctx.enter_context(tc.tile_pool(name="psTM", bufs=2, space="PSUM"))
            rr = [0]

            def big_psum(dt=F32):
                rr[0] += 1
                pool = psW if rr[0] % 3 == 0 else psA
                return pool.tile([128, AL], dt, name="bps",
                                 tag="a" if pool is psA else "w")

            # --- batch-0 critical-path inputs first (gates first matmul) ---
            at0 = sh24.tile([128, KC, AL], F32R, tag="t24")
            for k in range(KC):
                nc.sync.dma_start(at0[:, k, :], ATd[0, k * 128:(k + 1) * 128, :])
            gw0 = sh24b.tile([128, KC, L], F32R, tag="t24b")
            for k in range(KC):
                nc.sync.dma_start(gw0[:, k, :], GWd[k * 128:(k + 1) * 128, :])

            # --- resident weights (DMAs for the big FC weights are issued
            # after batch-0 phase A so they don't steal bandwidth from the
            # critical-path AT/GW/PT loads) ---
            fcaw = singles.tile([128, K2, L], BF16)
            fcpw = singles.tile([128, K2, L], BF16)
            fcab = singles.tile([128, L], F32)
            fcpb = singles.tile([128, L], F32)
            def bcast128(d):
                a = d.ap()
                return bass.AP(tensor=a.tensor, offset=a.offset,
                               ap=[[0, 128]] + list(a.ap))
            nc.sync.dma_start(fcab[:], bcast128(FCABd))
            nc.sync.dma_start(fcpb[:], bcast128(FCPBd))
            gb = singles.tile([128, KC], F32)
            nc.sync.dma_start(gb[:], GBd.ap().rearrange("(c p) -> p c", p=128))
            id32f = singles.tile([128, 128], F32)
            make_identity(nc, id32f[:])
            id32 = singles.tile([128, 128], F32R)
            nc.vector.tensor_copy(id32[:], id32f[:])
            id16 = singles.tile([128, 128], BF16)
            make_identity(nc, id16[:])

            for b in range(BPC):
                # ---------- Phase A: AgT[m, a] = (G_w.T @ A.T) + G_b ----------
                if b == 0:
                    at, gw = at0, gw0
                else:
                    at = sh24.tile([128, KC, AL], F32R, tag="t24")
                    for k in range(KC):
                        nc.sync.dma_start(at[:, k, :], ATd[b, k * 128:(k + 1) * 128, :])
                    gw = sh24b.tile([128, KC, L], F32R, tag="t24b")
                    for k in range(KC):
                        nc.sync.dma_start(gw[:, k, :], GWd[k * 128:(k + 1) * 128, :])

                agt = perb.tile([128, KC, AL], F32R, tag="agt")
                for mi in range(KC):
                    pw = big_psum()
                    for h in range(2):
                        for k in range(KC):
                            nc.tensor.matmul(
                                pw[:, h * 512:(h + 1) * 512],
                                gw[:, k, mi * 128:(mi + 1) * 128],
                                at[:, k, h * 512:(h + 1) * 512],
                                start=(k == 0), stop=(k == KC - 1))
                    # add G_b (per-partition) while copying PSUM -> SBUF
                    nc.scalar.add(agt[:, mi, :], pw[:], gb[:, mi:mi + 1])

                if b == 0:
                    for k in range(K2):
                        nc.sync.dma_start(fcpw[:, k, :], FCPWd[k * 128:(k + 1) * 128, :])
                        nc.sync.dma_start(fcaw[:, k, :], FCAWd[k * 128:(k + 1) * 128, :])

                # ---------- Phase B: Ag[a, m] bf16 via PE transpose ----------
                ag = perb.tile([128, ACH, L], BF16, tag="ag")
                for ai in range(ACH):
                    pt_ = big_psum(F32R)
                    for mi in range(KC):
                        nc.tensor.transpose(
                            pt_[:, mi * 128:(mi + 1) * 128],
                            agt[:, mi, ai * 128:(ai + 1) * 128], id32[:])
                    nc.vector.tensor_copy(ag[:, ai, :], pt_[:, :L])

                # ---------- Phase C: stream over p blocks ----------
                # Software-pipelined (lag-1): the W+softmax of block pb is
                # emitted before the consume-tail (transpose/MPT/fcp/MAT) of
                # block pb-1 so the PE queue never waits on a fresh softmax.
                mat = sh24.tile([128, KC, AL], F32R, tag="t24")
                saved = {}
                for pb in range(NPB + 1):
                    if pb < NPB:
                        ptb = blk2.tile([128, KC, PB], F32R, tag="ptb")
                        for k in range(KC):
                            nc.sync.dma_start(
                                ptb[:, k, :],
                                PTd[b, k * 128:(k + 1) * 128, pb * PB:(pb + 1) * PB])
                        pnb = blk2.tile([128, PBC, L], BF16, tag="pnb")
                        for pi in range(PBC):
                            p0 = pb * PB + pi * 128
                            nc.sync.dma_start(pnb[:, pi, :], PNd[b, p0:p0 + 128, :])

                        wn = blk2.tile([128, PBC, AL], BF16, tag="wn")
                        # scores + softmax per p-chunk
                        for pi in range(PBC):
                            pw = big_psum()
                            for h in range(2):
                                for k in range(KC):
                                    nc.tensor.matmul(
                                        pw[:, h * 512:(h + 1) * 512],
                                        ptb[:, k, pi * 128:(pi + 1) * 128],
                                        agt[:, k, h * 512:(h + 1) * 512],
                                        start=(k == 0), stop=(k == KC - 1))
                            nm0 = stats.tile([128, 1], F32, tag="nm0")
                            nm1 = stats.tile([128, 1], F32, tag="nm1")
                            nc.vector.tensor_reduce(
                                nm0[:], pw[:, 0:512], axis=mybir.AxisListType.X,
                                op=mybir.AluOpType.max, negate=True)
                            nc.vector.tensor_reduce(
                                nm1[:], pw[:, 512:AL], axis=mybir.AxisListType.X,
                                op=mybir.AluOpType.max, negate=True)
                            negmax = stats.tile([128, 1], F32, tag="negmax")
                            nc.vector.tensor_tensor(
                                negmax[:], nm0[:], nm1[:], op=mybir.AluOpType.min)
                            rsum = stats.tile([128, 1], F32, tag="rsum")
                            nc.scalar.activation(
                                wn[:, pi, :], pw[:],
                                mybir.ActivationFunctionType.Exp,
                                bias=negmax[:], scale=1.0, accum_out=rsum[:])
                            rinv = stats.tile([128, 1], F32, tag="rinv")
                            nc.vector.reciprocal(rinv[:], rsum[:])
                            nc.vector.tensor_scalar_mul(wn[:, pi, :], wn[:, pi, :], rinv[:])
                        saved[pb] = (ptb, pnb, wn)

                    if pb < 1:
                        continue
                    q = pb - 1
                    ptb, pnb, wn = saved.pop(q)

                    # transpose Wn -> WnT [a, p_block]
                    wnt = blk1.tile([128, ACH, PB], BF16, tag="wnt")
                    for ai in range(ACH):
                        ptr = psTM.tile([128, PB], BF16, tag="tm")
                        for pi in range(PBC):
                            nc.tensor.transpose(
                                ptr[:, pi * 128:(pi + 1) * 128],
                                wn[:, pi, ai * 128:(ai + 1) * 128], id16[:])
                        nc.vector.tensor_copy(wnt[:, ai, :], ptr[:])

                    # MPT[m, p_blk] = Ag.T @ WnT ; SPcatT rows
                    spcat = blk1.tile([128, K2, PB], BF16, tag="spcat")
                    for mi in range(KC):
                        pm = psTM.tile([128, PB], F32, tag="tm")
                        for ai in range(ACH):
                            nc.tensor.matmul(
                                pm[:], ag[:, ai, mi * 128:(mi + 1) * 128],
                                wnt[:, ai, :],
                                start=(ai == 0), stop=(ai == ACH - 1))
                        nc.vector.tensor_sub(spcat[:, mi, :], pm[:], ptb[:, mi, :])
                        nc.vector.tensor_mul(spcat[:, KC + mi, :], pm[:], ptb[:, mi, :])

                    # SP_out rows for this block
                    for pi in range(PBC):
                        ps = big_psum()
                        for h, (n0, n1) in enumerate(((0, 512), (512, L))):
                            for k in range(K2):
                                nc.tensor.matmul(
                                    ps[:, n0:n1],
                                    spcat[:, k, pi * 128:(pi + 1) * 128],
                                    fcpw[:, k, n0:n1],
                                    start=(k == 0), stop=(k == K2 - 1))
                        so = stage.tile([128, L], F32, tag="st")
                        nc.vector.tensor_add(so[:], ps[:, :L], fcpb[:])
                        nc.scalar.activation(so[:], so[:],
                                             mybir.ActivationFunctionType.Relu)
                        p0 = q * PB + pi * 128
                        nc.scalar.dma_start(SPd[b, p0:p0 + 128, :], so[:])

                    # MAT partial: mat[l, a] += P_blk.T-contract @ Wn_blk
                    for li in range(KC):
                        pa = big_psum()
                        for h in range(2):
                            for pi in range(PBC):
                                nc.tensor.matmul(
                                    pa[:, h * 512:(h + 1) * 512],
                                    pnb[:, pi, li * 128:(li + 1) * 128],
                                    wn[:, pi, h * 512:(h + 1) * 512],
                                    start=(pi == 0), stop=(pi == PBC - 1))
                        if q == 0:
                            nc.vector.tensor_copy(mat[:, li, :], pa[:])
                        else:
                            nc.vector.tensor_add(mat[:, li, :], pa[:], mat[:, li, :])

                # ---------- Phase D: SAcatT + fca ----------
                sacat = sh24b.tile([128, K2, AL], BF16, tag="t24b")
                for li in range(KC):
                    nc.vector.tensor_sub(sacat[:, li, :], mat[:, li, :], agt[:, li, :])
                    nc.vector.tensor_mul(sacat[:, KC + li, :], mat[:, li, :], agt[:, li, :])

                for ai in range(ACH):
                    ps = big_psum()
                    for h, (n0, n1) in enumerate(((0, 512), (512, L))):
                        for k in range(K2):
                            nc.tensor.matmul(
                                ps[:, n0:n1],
                                sacat[:, k, ai * 128:(ai + 1) * 128],
                                fcaw[:, k, n0:n1],
                                start=(k == 0), stop=(k == K2 - 1))
                    so = stage.tile([128, L], F32, tag="st")
                    nc.vector.tensor_add(so[:], ps[:, :L], fcab[:])
                    nc.scalar.activation(so[:], so[:],
                                         mybir.ActivationFunctionType.Relu)
                    nc.scalar.dma_start(SAd[b, ai * 128:(ai + 1) * 128, :], so[:])

    nc.compile()
    return nc


_NC_CACHE = {}


def _get_program():
    if "nc" not in _NC_CACHE:
        _NC_CACHE["nc"] = build_program()
    return _NC_CACHE["nc"]


def kernel(A, P, G_w, G_b, fca_w, fca_b, fcp_w, fcp_b):
    A = np.asarray(A, dtype=np.float32)
    P = np.asarray(P, dtype=np.float32)
    G_w = np.ascontiguousarray(np.asarray(G_w, dtype=np.float32))
    G_b = np.ascontiguousarray(np.asarray(G_b, dtype=np.float32))
    fca_w16 = np.asarray(fca_w, dtype=np.float32).astype(ml_dtypes.bfloat16)
    fcp_w16 = np.asarray(fcp_w, dtype=np.float32).astype(ml_dtypes.bfloat16)
    fca_b = np.ascontiguousarray(np.asarray(fca_b, dtype=np.float32))
    fcp_b = np.ascontiguousarray(np.asarray(fcp_b, dtype=np.float32))

    AT = np.ascontiguousarray(A.transpose(0, 2, 1))      # [16, 768, 1024]
    PT = np.ascontiguousarray(P.transpose(0, 2, 1))      # [16, 768, 2048]
    PN = P.astype(ml_dtypes.bfloat16)                     # [16, 2048, 768]

    nc = _get_program()
    in_maps = []
    for c in range(NCORES):
        b0, b1 = c * BPC, (c + 1) * BPC
        in_maps.append({
            "AT": np.ascontiguousarray(AT[b0:b1]),
            "PT": np.ascontiguousarray(PT[b0:b1]),
            "PN": np.ascontiguousarray(PN[b0:b1]),
            "GW": G_w, "GB": G_b,
            "FCAW": fca_w16, "FCAB": fca_b,
            "FCPW": fcp_w16, "FCPB": fcp_b,
        })
    res = bass_utils.run_bass_kernel_spmd(nc, in_maps, core_ids=list(range(NCORES)))
    SA = np.concatenate([r["SA"] for r in res.results], axis=0)
    SP = np.concatenate([r["SP"] for r in res.results], axis=0)
    return (SA, SP)
